# revision 11
# baseline (speedup 1.0000x reference)
"""Pixelwise contrastive loss on 8 Trainium2 cores.

Phase A (per core k): indirect-gather the pixel embeddings whose b==k from
the [C,H,W] map slice, L2-normalize, emit bf16 [128, NPAD] (sample-major
partitions). Host glue reassembles the global [C, 10240] normalized matrix
(the "all-gather"). Phase B (per core k): 256 pos rows x 10240 cols of
cosine similarity via PE matmul, exp + row-sum on ACT, NLL tail. Host sums
the 8 partial log-likelihood tiles into the scalar loss.
"""

import sys

if "/opt/trn_rl_repo" not in sys.path:
    sys.path.insert(0, "/opt/trn_rl_repo")

import numpy as np
import ml_dtypes

from concourse import bass, mybir, bass_utils
from concourse import bacc
import concourse.tile as tile

B, C, H, W = 8, 128, 256, 256
HW = H * W
N_POS, N_NEG = 2048, 8192
NTOT = N_POS + N_NEG
NCORES = 8
BF16 = ml_dtypes.bfloat16
E1 = float(np.exp(np.float32(1.0)))

_PROG_A = {}
_PROG_B = None


def _build_phase_a(NT):
    NPAD = NT * 128
    nc = bacc.Bacc("TRN2", target_bir_lowering=False)
    mapk = nc.dram_tensor("mapk", [C * HW, 1], mybir.dt.float32, kind="ExternalInput")
    tblT = nc.dram_tensor("tbl", [128, NPAD], mybir.dt.int32, kind="ExternalInput")
    xnT = nc.dram_tensor("xn", [128, NPAD], mybir.dt.bfloat16, kind="ExternalOutput")
    with tile.TileContext(nc) as tc:
        with tc.tile_pool(name="main", bufs=1) as pool, \
             tc.tile_pool(name="gt", bufs=NT) as pool_g, \
             tc.tile_pool(name="sq", bufs=NT) as pool_sq:
            tbl_s = pool.tile([128, NPAD], mybir.dt.int32)
            nc.sync.dma_start(out=tbl_s[:], in_=tblT[:])
            n2 = pool.tile([128, NT], mybir.dt.float32)
            xg = []
            for t in range(NT):
                g = pool_g.tile([128, 128], mybir.dt.float32)
                xg.append(g)
                nc.gpsimd.indirect_dma_start(
                    out=g[:],
                    out_offset=None,
                    in_=mapk[:],
                    in_offset=bass.IndirectOffsetOnAxis(
                        ap=tbl_s[:, t * 128:(t + 1) * 128], axis=0
                    ),
                )
                sq = pool_sq.tile([128, 128], mybir.dt.float32)
                nc.scalar.activation(
                    out=sq[:], in_=g[:],
                    func=mybir.ActivationFunctionType.Square,
                    accum_out=n2[:, t:t + 1],
                )
            r1 = pool.tile([128, NT], mybir.dt.float32)
            r2 = pool.tile([128, NT], mybir.dt.float32)
            r3 = pool.tile([128, NT], mybir.dt.float32)
            nc.vector.reciprocal(out=r1[:], in_=n2[:])
            nc.scalar.activation(
                out=r2[:], in_=r1[:], func=mybir.ActivationFunctionType.Sqrt
            )
            # x / max(norm, 1e-6) == x * min(1/norm, 1e6)
            nc.vector.tensor_scalar_min(out=r3[:], in0=r2[:], scalar1=1.0e6)
            xn = pool.tile([128, NPAD], mybir.dt.bfloat16)
            for t in range(NT):
                # On ACT (not DVE): gather dep already implied by Square t's
                # wait, so only t=0 carries a (single) DVE wait for r3.
                nc.scalar.activation(
                    out=xn[:, t * 128:(t + 1) * 128],
                    in_=xg[t][:],
                    func=mybir.ActivationFunctionType.Copy,
                    scale=r3[:, t:t + 1],
                )
            nc.sync.dma_start(out=xnT[:], in_=xn[:])
    nc.finalize()
    return nc


def _build_phase_b():
    NCH = 20  # 10240 / 512 chunks; chunks 0..3 are pos columns, 4..19 neg
    nc = bacc.Bacc("TRN2", target_bir_lowering=False)
    # allr = [allN | this core's 256 pos rows] concatenated on the free dim so
    # ONE DMA covers both and every matmul carries at most one sem wait.
    allrT = nc.dram_tensor(
        "allr", [128, NTOT + 256], mybir.dt.bfloat16, kind="ExternalInput"
    )
    llT = nc.dram_tensor("ll", [128, 2], mybir.dt.float32, kind="ExternalOutput")
    with tile.TileContext(nc) as tc:
        with tc.tile_pool(name="main", bufs=1) as pool, \
             tc.tile_pool(name="ps", bufs=4, space="PSUM") as pool_ps, \
             tc.tile_pool(name="es", bufs=2 * NCH) as pool_es:
            allr_s = pool.tile([128, NTOT + 256], mybir.dt.bfloat16)
            nc.sync.dma_start(out=allr_s[:], in_=allrT[:])
            sums = pool.tile([128, 2 * NCH], mybir.dt.float32)
            for t in range(NCH):
                for g in range(2):
                    ps = pool_ps.tile([128, 512], mybir.dt.float32)
                    nc.tensor.matmul(
                        out=ps[:],
                        lhsT=allr_s[:, NTOT + g * 128:NTOT + (g + 1) * 128],
                        rhs=allr_s[:, t * 512:(t + 1) * 512],
                        start=True,
                        stop=True,
                    )
                    es = pool_es.tile([128, 512], mybir.dt.float32)
                    nc.scalar.activation(
                        out=es[:], in_=ps[:],
                        func=mybir.ActivationFunctionType.Exp,
                        accum_out=sums[:, g * NCH + t:g * NCH + t + 1],
                    )
            possum = pool.tile([128, 2], mybir.dt.float32)
            negsum = pool.tile([128, 2], mybir.dt.float32)
            for g in range(2):
                nc.vector.tensor_reduce(
                    out=possum[:, g:g + 1], in_=sums[:, g * NCH:g * NCH + 4],
                    axis=mybir.AxisListType.X, op=mybir.AluOpType.add,
                )
                nc.vector.tensor_reduce(
                    out=negsum[:, g:g + 1], in_=sums[:, g * NCH + 4:(g + 1) * NCH],
                    axis=mybir.AxisListType.X, op=mybir.AluOpType.add,
                )
            pnum = pool.tile([128, 2], mybir.dt.float32)
            nc.vector.tensor_scalar_add(out=pnum[:], in0=possum[:], scalar1=-E1)
            den = pool.tile([128, 2], mybir.dt.float32)
            nc.vector.tensor_tensor(
                out=den[:], in0=pnum[:], in1=negsum[:], op=mybir.AluOpType.add
            )
            rden = pool.tile([128, 2], mybir.dt.float32)
            nc.vector.reciprocal(out=rden[:], in_=den[:])
            lik = pool.tile([128, 2], mybir.dt.float32)
            nc.vector.tensor_tensor(
                out=lik[:], in0=pnum[:], in1=rden[:], op=mybir.AluOpType.mult
            )
            lls = pool.tile([128, 2], mybir.dt.float32)
            nc.scalar.activation(
                out=lls[:], in_=lik[:], func=mybir.ActivationFunctionType.Ln
            )
            nc.sync.dma_start(out=llT[:], in_=lls[:])
    nc.finalize()
    return nc


def _get_out(core_results, key):
    if key in core_results:
        return np.asarray(core_results[key])
    return np.asarray(next(iter(core_results.values())))


def _run_all(inputs, trace=False):
    global _PROG_B
    psm = np.ascontiguousarray(np.asarray(inputs["predict_seg_map"], dtype=np.float32))
    pb = np.asarray(inputs["pos_b"]).astype(np.int64)
    ph = np.asarray(inputs["pos_h"]).astype(np.int64)
    pw = np.asarray(inputs["pos_w"]).astype(np.int64)
    nb = np.asarray(inputs["neg_b"]).astype(np.int64)
    nh = np.asarray(inputs["neg_h"]).astype(np.int64)
    nw = np.asarray(inputs["neg_w"]).astype(np.int64)

    allb = np.concatenate([pb, nb])
    allpix = np.concatenate([ph * W + pw, nh * W + nw])
    gids = np.arange(NTOT, dtype=np.int64)

    ids_per, pix_per = [], []
    for k in range(NCORES):
        m = allb == k
        idk, pxk = gids[m], allpix[m]
        o = np.argsort(pxk, kind="stable")
        ids_per.append(idk[o])
        pix_per.append(pxk[o])
    nmax = max(len(x) for x in ids_per)
    NT = (nmax + 127) // 128
    NPAD = NT * 128

    coff = np.arange(C, dtype=np.int64)[None, None, :] * HW
    tbls = []
    for k in range(NCORES):
        e = np.zeros(NPAD, np.int64)
        e[:len(pix_per[k])] = pix_per[k]
        e2 = e.reshape(NT, 128)  # [t, p]
        tbl = e2.T[:, :, None] + coff  # [p, t, c]
        tbls.append(np.ascontiguousarray(tbl.reshape(128, NPAD).astype(np.int32)))

    if NT not in _PROG_A:
        _PROG_A[NT] = _build_phase_a(NT)
    nc_a = _PROG_A[NT]
    in_maps_a = [
        {"mapk": np.ascontiguousarray(psm[k].reshape(-1, 1)), "tbl": tbls[k]}
        for k in range(NCORES)
    ]
    ra = bass_utils.run_bass_kernel_spmd(
        nc_a, in_maps_a, list(range(NCORES)), trace=trace
    )

    allN_T = np.zeros((NTOT, C), dtype=BF16)
    for k in range(NCORES):
        xnk = _get_out(ra.results[k], "xn")  # [128, NPAD]
        nk = len(ids_per[k])
        v = xnk.reshape(128, NT, 128).transpose(1, 0, 2).reshape(NPAD, 128)[:nk]
        allN_T[ids_per[k]] = v
    allN = np.ascontiguousarray(allN_T.T)  # [C, NTOT]

    if _PROG_B is None:
        _PROG_B = _build_phase_b()
    in_maps_b = [
        {
            "allr": np.ascontiguousarray(
                np.concatenate([allN, allN[:, k * 256:(k + 1) * 256]], axis=1)
            )
        }
        for k in range(NCORES)
    ]
    rb = bass_utils.run_bass_kernel_spmd(
        _PROG_B, in_maps_b, list(range(NCORES)), trace=trace
    )

    tot = 0.0
    for k in range(NCORES):
        tot += float(_get_out(rb.results[k], "ll").astype(np.float64).sum())
    nll = -tot / N_POS

    ns = None
    if trace:
        ns = (ra.exec_time_ns or 0) + (rb.exec_time_ns or 0)
    return np.float32(nll), ns


def kernel(predict_seg_map, pos_b, pos_h, pos_w, neg_b, neg_h, neg_w):
    out, _ = _run_all(
        {
            "predict_seg_map": predict_seg_map,
            "pos_b": pos_b, "pos_h": pos_h, "pos_w": pos_w,
            "neg_b": neg_b, "neg_h": neg_h, "neg_w": neg_w,
        },
        trace=False,
    )
    return np.asarray(out, dtype=np.float32)
